# revision 16
# baseline (speedup 1.0000x reference)
"""Trainium2 Bass kernel for nn_EquivarientScalar (segment_reduce).

Computation (reference): 2 stacked GatedEquivariant layers over N=100000
atoms (pointwise per atom), then sc = s @ out_w + out_b and a masked
segment-sum y[b] = sum_n sc[n] * batch_mask[b, n].

v2 strategy (vs the fp32r baseline):
  - Everything bf16: inputs, weights, intermediates. Halves HBM traffic
    (memory-target regime) at identical PE speed (1 cyc/row, same as
    fp32r at N>=256). Matmul accumulation stays fp32 in PSUM.
  - 12800 atoms/core (25 blocks of 512; 12 pairs + 1 tail block) instead
    of 13312 -- 4% less padding work everywhere.
  - Layer-2 scalar path folded on host: Wp = a2w_s0 @ a1w_s1 replaces
    the a2s matmul + bias evac; its bias lands in the layer-2 silu bias.
  - v-matmuls run once per block at N=1536 (all 3 spatial planes) into a
    3-bank PSUM tile; the gate is ONE 1536-wide DVE op via broadcast AP.
  - Norm chain split across engines by measured throughput:
      ACT   squares planes c0,c1 (PSUM->SBUF bf16)      [1 op/block]
      Pool  q01 = sq0 + sq1 (SBUF bf16, pair-wide)      [1 op/pair]
      DVE   q = c2^2 + q01 (custom op, 1 PSUM stream)   [1 op/block]
      Pool  rsqrt seed via u32 value-cast bit trick     [1 op/pair]
      DVE   n2 = (q*y0)*(C0-C1*(q*y0)*y0) fused Newton  [1 op/pair]
    (Pool/GPSIMD cannot touch PSUM, so all PSUM reads sit on ACT/DVE.)
  - Segment reduce on-chip as before: sc columns via h2-chunk stationary
    matmuls, y += maskT_chunk^T @ sc per 128 atoms, mask carried in bf16
    (0/1 exact). Host sums the 8 per-core partial y vectors.
"""

import os
import sys

for _p in ("/opt/trn_rl_repo", "/root/.axon_site/_ro/trn_rl_repo"):
    if os.path.isdir(_p) and _p not in sys.path:
        sys.path.insert(0, _p)

os.environ.setdefault("BASS_NEVER_TRACE", "1")  # no NTFF hook in this axon build

import numpy as np

import concourse.bass as bass
import concourse.tile as tile
from concourse import bacc, mybir
from concourse import dve_ops as _dve_ops
from concourse.alu_op_type import AluOpType
from concourse.bass_utils import run_bass_kernel_spmd
from concourse.dve_ops import OPS as _DVE_OPS
from concourse.dve_ops import _CUSTOM_DVE_ROW_BASE, _SUB_OPCODE_FOR_NAME, DveOp
from concourse.dve_spec import C0 as _C0
from concourse.dve_spec import C1 as _C1
from concourse.dve_spec import Spec as _Spec
from concourse.dve_spec import Src0 as _Src0
from concourse.dve_spec import Src1 as _Src1
from concourse.dve_spec import lower as _dve_lower
from concourse.dve_spec import sq as _sq
from concourse.dve_uop import DveOpSpec as _DveOpSpec

N_CORES = 8
NA_FULL = 100000
NA_CORE = NA_FULL // N_CORES   # 12500
BLK = 512
NBLK = 25                      # 25 blocks of 512 = 12800 padded atoms/core
NA = NBLK * BLK
F = 128

F32 = mybir.dt.float32
BF16 = mybir.dt.bfloat16
U32 = mybir.dt.uint32
AF = mybir.ActivationFunctionType

W_NAMES = ["w1_0", "w2_0", "w2_1", "a1w_s0", "a1w_n0", "a1w_n1",
           "a2w_g0", "wp"]

# rsqrt magic seed (computed via u32 value-casts on Pool) + one fused
# Newton-ish stage on DVE. Constants fitted offline (baseline-validated):
# wide-range fp32 max rel err ~1e-3; q=0 -> 0 (no NaN).
MAGIC_F = 1596013007.0
SQ1_C0, SQ1_C1 = 1.6695484, 0.688087555  # n2 = (q*y0)*(C0 - C1*q*y0^2)

_last_results = None
_last_nc = None
_last_in_maps = None
ABLATE = "full"  # timing ablations: full | no_scy | no_gate | no_norm | no_mid


def _ref_sqa(in0, in1, s0, s1, imm2):
    return (in0.astype(np.float32) * in0 + in1).astype(np.float32)


def _ref_sqrt_fin(in0, in1, s0, s1, imm2):
    qy = (in0 * in1).astype(np.float32)
    return (qy * (np.float32(s0) - np.float32(s1) * (qy * in1))).astype(np.float32)


def _register_ops():
    by_name = {op.name: op for op in _DVE_OPS}
    if "SQA_ANT" in _SUB_OPCODE_FOR_NAME and "SQRT_FIN_ANT" in _SUB_OPCODE_FOR_NAME:
        return by_name["SQA_ANT"], by_name["SQRT_FIN_ANT"]

    def make(name, body, ref):
        if name in _SUB_OPCODE_FOR_NAME:
            return by_name[name]
        op = DveOp(name, _Spec(body=body, reference=ref), subdim=False,
                   uops_sha={})
        opcode = _CUSTOM_DVE_ROW_BASE + len(_DVE_OPS)
        for ver in ("v3", "v4"):
            try:
                spec = _DveOpSpec(name=name, opcode=opcode,
                                  uops=_dve_lower(op.spec, ver=ver),
                                  rd1_en=_dve_ops.has_src1(op.spec))
                op.uops_sha[ver] = spec.sha(ver)
            except Exception:
                pass
        _SUB_OPCODE_FOR_NAME[name] = opcode
        _DVE_OPS.append(op)
        return op

    # q = c2^2 + q01        [in0 = v2_c2 (PSUM), in1 = q01 (SBUF)]
    sqa = make("SQA_ANT", _sq(_Src0) + _Src1, _ref_sqa)
    # n2 = (q*y0) * (C0 - C1 * (q*y0) * y0)   [in0 = q, in1 = y0 seed]
    _qy = _Src0 * _Src1
    fin = make("SQRT_FIN_ANT",
               _qy * (_C0 - _C1 * (_qy * _Src1)),
               _ref_sqrt_fin)
    return sqa, fin


def _build(b_fold: float, reps: int = 1, trace_sim: bool = False):
    # reps > 1 repeats the whole computation inside one NEFF (timing only --
    # y then accumulates reps copies; used to subtract host/tunnel overhead).
    OP_SQA, OP_FIN = _register_ops()
    nc = bacc.Bacc("TRN2", target_bir_lowering=False)

    # packed input, partition-major: row f = [s 512 | v 3*512 | mask 4*128]
    xp = nc.dram_tensor("xpack", (F, NBLK, 5 * BLK), BF16, kind="ExternalInput")
    wd = {n: nc.dram_tensor(n, (F, F), BF16, kind="ExternalInput") for n in W_NAMES}
    # (F, 2): col 0 = a2w_s1 @ out_w, col 1 = zeros
    wf = nc.dram_tensor("w_fold", (F, 2), BF16, kind="ExternalInput")
    a1b0 = nc.dram_tensor("a1b0", (F, 1), F32, kind="ExternalInput")
    b2p = nc.dram_tensor("b2p", (F, 1), F32, kind="ExternalInput")
    a2bg = nc.dram_tensor("a2bg", (F, 1), F32, kind="ExternalInput")
    y = nc.dram_tensor("y", (F, 1), F32, kind="ExternalOutput")

    # 12 pairs + tail block
    groups = [[2 * i, 2 * i + 1] for i in range(12)] + [[24]]

    with tile.TileContext(nc, trace_sim=trace_sim) as tc:
        # PSUM budget (8 banks): v3 tag 3 banks x2 bufs + "a" tag 1 bank x2.
        with tc.tile_pool(name="wpool", bufs=1) as wp, \
             tc.tile_pool(name="io", bufs=6) as io, \
             tc.tile_pool(name="work", bufs=2) as wk, \
             tc.tile_pool(name="workx", bufs=5) as wkx, \
             tc.tile_pool(name="psv", bufs=2, space="PSUM") as psv, \
             tc.tile_pool(name="psa", bufs=2, space="PSUM") as psa:

            wt = {}
            for n in W_NAMES:
                wt[n] = wp.tile([F, F], BF16, name=n, tag=n)
                nc.sync.dma_start(out=wt[n], in_=wd[n][:, :])
            wft = wp.tile([F, 2], BF16, tag="wf")
            nc.sync.dma_start(out=wft, in_=wf[:, :])
            bt = {}
            for n, d in [("a1b0", a1b0), ("b2p", b2p), ("a2bg", a2bg)]:
                bt[n] = wp.tile([F, 1], F32, name=n, tag=n)
                nc.sync.dma_start(out=bt[n], in_=d[:, :])

            y_sb = wp.tile([F, 1], F32, tag="y_sb")
            nc.vector.memset(y_sb, 0.0)

            def norm_front(xts_or_vouts, nh, tag, from_xt):
                """v-matmuls + squares for nh blocks; returns (v3s, q, n2P).
                Emits: per h: 1 matmul (N=1536) + 1 ACT square (c01) +
                1 DVE SQA; per group: 1 Pool add, 1 Pool seed, 1 DVE fin."""
                w = wt["w2_0"] if tag == "l1" else wt["w2_1"]
                sqP = wk.tile([F, nh, 2, BLK], BF16, tag=f"sqP_{tag}")
                v3s = []
                for h in range(nh):
                    v3 = psv.tile([F, 3, BLK], F32, tag="v3")
                    for c in range(3):
                        if from_xt:
                            rhs = xts_or_vouts[:, h,
                                               (1 + c) * BLK:(2 + c) * BLK]
                        else:
                            rhs = xts_or_vouts[h][:, c, :]
                        nc.tensor.matmul(v3[:, c, :], w, rhs)
                    if ABLATE != "no_norm":
                        nc.scalar.activation(out=sqP[:, h, :, :],
                                             in_=v3[:, 0:2, :], func=AF.Square)
                    v3s.append(v3)
                if ABLATE == "no_norm":
                    n2P = wk.tile([F, nh, BLK], BF16, tag=f"n2P_{tag}")
                    nc.vector.memset(n2P, 0.5)
                    return v3s, n2P
                q01P = wk.tile([F, nh, BLK], BF16, tag=f"q01P_{tag}")
                nc.vector.tensor_tensor(out=q01P, in0=sqP[:, :, 0, :],
                                        in1=sqP[:, :, 1, :], op=AluOpType.add)
                q = wk.tile([F, nh, BLK], F32, tag=f"q_{tag}")
                for h in range(nh):
                    nc.vector._custom_dve(OP_SQA, out=q[:, h, :],
                                          in0=v3s[h][:, 2, :],
                                          in1=q01P[:, h, :])
                sd = wk.tile([F, nh, BLK], F32, tag=f"sd_{tag}")
                nc.gpsimd.tensor_scalar(out=sd.bitcast(U32),
                                        in0=q.bitcast(U32),
                                        scalar1=-0.5, scalar2=MAGIC_F,
                                        op0=AluOpType.mult, op1=AluOpType.add)
                n2P = wkx.tile([F, nh, BLK], BF16, tag=f"n2P_{tag}")
                nc.vector._custom_dve(OP_FIN, out=n2P, in0=q, in1=sd,
                                      s0=SQ1_C0, s1=SQ1_C1)
                return v3s, n2P

            # Manual 3-stage software pipeline: iteration i emits
            # front(i) | mid(i-1) | tail(i-2), so each engine always has
            # independent work from adjacent pairs in its in-order queue.
            ctxs = {}

            def st_front(it):
                hs = groups[it % len(groups)]
                nh = len(hs)
                b0 = hs[0]
                xt = io.tile([F, nh, 5 * BLK], BF16, tag="xt")
                with tc.high_priority(offset=110):
                    nc.sync.dma_start(out=xt, in_=xp[:, b0:b0 + nh, :])
                _, n2P1 = norm_front(xt, nh, "l1", True)
                ctxs[it] = {"nh": nh, "xt": xt, "n2P1": n2P1}

            def st_mid(it):
                cx = ctxs[it]
                nh, xt, n2P1 = cx["nh"], cx["xt"], cx["n2P1"]
                if ABLATE == "no_mid":
                    h1P = wkx.tile([F, nh, BLK], BF16, tag="h1P")
                    nc.vector.memset(h1P, 0.5)
                    n2P2 = wkx.tile([F, nh, BLK], BF16, tag="n2P_l2")
                    nc.vector.memset(n2P2, 0.5)
                    cx["h1P"], cx["n2P2"] = h1P, n2P2
                    return
                h1P = wkx.tile([F, nh, BLK], BF16, tag="h1P")
                gP = wk.tile([F, nh, BLK], BF16, tag="gP")
                vouts = []
                for h in range(nh):
                    a1 = psa.tile([F, BLK], F32, tag="a")
                    nc.tensor.matmul(a1, wt["a1w_s0"], xt[:, h, 0:BLK],
                                     start=True, stop=False)
                    nc.tensor.matmul(a1, wt["a1w_n0"], n2P1[:, h, :],
                                     start=False, stop=True)
                    nc.scalar.activation(out=h1P[:, h, :], in_=a1,
                                         func=AF.Silu, bias=bt["a1b0"])

                    a2g = psa.tile([F, BLK], F32, tag="a")
                    nc.tensor.matmul(a2g, wt["a2w_g0"], h1P[:, h, :])
                    nc.scalar.activation(out=gP[:, h, :], in_=a2g,
                                         func=AF.Identity, bias=bt["a2bg"])

                    # v1 matmuls + one 1536-wide gate via broadcast AP
                    v13 = psv.tile([F, 3, BLK], F32, tag="v3")
                    for c in range(3):
                        nc.tensor.matmul(v13[:, c, :], wt["w1_0"],
                                         xt[:, h, (1 + c) * BLK:(2 + c) * BLK])
                    g = gP[:, h, :]
                    g_bc3 = bass.AP(tensor=g.tensor, offset=g.offset,
                                    ap=[g.ap[0], [0, 3], g.ap[1]])
                    vout = wk.tile([F, 3, BLK], BF16, tag="vout")
                    if ABLATE == "no_gate":
                        nc.vector.memset(vout, 0.5)
                    else:
                        nc.vector.tensor_tensor(out=vout, in0=v13, in1=g_bc3,
                                                op=AluOpType.mult)
                    vouts.append(vout)

                _, n2P2 = norm_front(vouts, nh, "l2", False)
                cx["h1P"], cx["n2P2"] = h1P, n2P2

            def st_tail(it):
                cx = ctxs.pop(it)
                nh, xt, h1P, n2P2 = cx["nh"], cx["xt"], cx["h1P"], cx["n2P2"]
                h2P = wk.tile([F, nh, BLK], BF16, tag="h2P")
                for h in range(nh):
                    a1b_ = psa.tile([F, BLK], F32, tag="a")
                    nc.tensor.matmul(a1b_, wt["wp"], h1P[:, h, :],
                                     start=True, stop=False)
                    nc.tensor.matmul(a1b_, wt["a1w_n1"], n2P2[:, h, :],
                                     start=False, stop=True)
                    nc.scalar.activation(out=h2P[:, h, :], in_=a1b_,
                                         func=AF.Silu, bias=bt["b2p"])

                if ABLATE == "no_scy":
                    return
                sc_ps = psa.tile([F, nh, BLK // F, 2], F32, tag="a")
                for h in range(nh):
                    for k in range(BLK // F):
                        nc.tensor.matmul(sc_ps[:, h, k, :],
                                         h2P[:, h, k * F:(k + 1) * F], wft)
                sc_sb = wk.tile([F, nh, BLK // F, 2], BF16, tag="sc_sb")
                nc.vector.tensor_scalar(out=sc_sb, in0=sc_ps,
                                        scalar1=float(b_fold),
                                        scalar2=None, op0=AluOpType.add)
                y_ps = psa.tile([F, 2], F32, tag="a")
                nmm = 0
                nmm_tot = nh * (BLK // F)
                for h in range(nh):
                    mT_t = xt[:, h, 4 * BLK:].rearrange("p (k g) -> p k g",
                                                        k=BLK // F)
                    for k in range(BLK // F):
                        nc.tensor.matmul(y_ps, mT_t[:, k, :],
                                         sc_sb[:, h, k, :],
                                         start=(nmm == 0),
                                         stop=(nmm == nmm_tot - 1),
                                         skip_group_check=True)
                        nmm += 1
                nc.vector.tensor_tensor(out=y_sb, in0=y_sb, in1=y_ps[:, 0:1],
                                        op=AluOpType.add)

            total = len(groups) * reps
            for it in range(total + 4):
                if it < total:
                    st_front(it)
                if 2 <= it < total + 2:
                    st_mid(it - 2)
                if it >= 4:
                    st_tail(it - 4)

            nc.sync.dma_start(out=y[:, :], in_=y_sb)

    nc.finalize()
    return nc


def kernel(s, v, r, batch_mask, w1, w2, a1w, a1b, a2w, a2b, out_w, out_b):
    global _last_results
    del r  # unused by the reference computation

    s = np.ascontiguousarray(np.asarray(s, dtype=np.float32)).reshape(NA_FULL, F)
    v = np.ascontiguousarray(np.asarray(v, dtype=np.float32)).reshape(NA_FULL, 3, F)
    batch_mask = np.ascontiguousarray(
        np.asarray(batch_mask, dtype=np.float32)).reshape(F, NA_FULL)
    w1 = np.asarray(w1, dtype=np.float64)
    w2 = np.asarray(w2, dtype=np.float64)
    a1w = np.asarray(a1w, dtype=np.float64)
    a1b = np.asarray(a1b, dtype=np.float64)
    a2w = np.asarray(a2w, dtype=np.float64)
    a2b = np.asarray(a2b, dtype=np.float64)
    out_w = np.asarray(out_w, dtype=np.float64)
    out_b = np.asarray(out_b, dtype=np.float64)
    assert w1.shape == (2, F, F), "kernel is specialized to L=2"

    bf16 = mybir.dt.np(BF16)

    # folded final projection: sc = h2 @ (a2w_s1 @ out_w) + b_fold
    w_fold = np.zeros((F, 2), dtype=np.float64)
    w_fold[:, 0:1] = a2w[1][:, :F] @ out_w
    b_fold = float(a2b[1][:F] @ out_w[:, 0] + out_b[0])

    # folded layer-2 scalar path: a1w_s1^T s1 = (a2w_s0 @ a1w_s1)^T h1 + const
    wp = a2w[0][:, :F] @ a1w[1][:F, :]
    b2p = a1b[1] + a1w[1][:F, :].T @ a2b[0][:F]

    weights = {
        "w1_0": w1[0], "w2_0": w2[0], "w2_1": w2[1],
        "a1w_s0": a1w[0][:F, :], "a1w_n0": a1w[0][F:, :],
        "a1w_n1": a1w[1][F:, :], "a2w_g0": a2w[0][:, F:], "wp": wp,
    }
    weights = {k: np.ascontiguousarray(a, dtype=bf16)
               for k, a in weights.items()}
    w_fold16 = np.ascontiguousarray(w_fold, dtype=bf16)
    bias_cols = {
        "a1b0": np.ascontiguousarray(a1b[0].reshape(F, 1), dtype=np.float32),
        "b2p": np.ascontiguousarray(b2p.reshape(F, 1), dtype=np.float32),
        "a2bg": np.ascontiguousarray(a2b[0][F:].reshape(F, 1),
                                     dtype=np.float32),
    }

    in_maps = []
    for c in range(N_CORES):
        sl = slice(c * NA_CORE, (c + 1) * NA_CORE)
        sT = np.zeros((F, NA), dtype=np.float32)
        sT[:, :NA_CORE] = s[sl].T
        vT = np.zeros((F, 3, NA), dtype=np.float32)
        vT[:, :, :NA_CORE] = v[sl].transpose(2, 1, 0)
        mT = np.zeros((NA, F), dtype=np.float32)
        mT[:NA_CORE] = batch_mask[:, sl].T
        xp = np.empty((F, NBLK, 5 * BLK), dtype=np.float32)
        xp[:, :, 0:BLK] = sT.reshape(F, NBLK, BLK)
        xp[:, :, BLK:4 * BLK] = (
            vT.reshape(F, 3, NBLK, BLK).transpose(0, 2, 1, 3)
            .reshape(F, NBLK, 3 * BLK))
        xp[:, :, 4 * BLK:] = (
            mT.reshape(NBLK, BLK // F, F, F).transpose(2, 0, 1, 3)
            .reshape(F, NBLK, BLK))
        m = {"xpack": np.ascontiguousarray(xp, dtype=bf16),
             "w_fold": w_fold16}
        m.update(weights)
        m.update(bias_cols)
        in_maps.append(m)

    nc = _build(b_fold)
    res = run_bass_kernel_spmd(nc, in_maps, core_ids=list(range(N_CORES)))
    global _last_nc, _last_in_maps
    _last_results, _last_nc, _last_in_maps = res, nc, in_maps

    yv = np.zeros((F, 1), dtype=np.float64)
    for c in range(N_CORES):
        yv += res.results[c]["y"].astype(np.float64)
    return yv.astype(np.float32)


# revision 23
# speedup vs baseline: 1.2194x; 1.2194x over previous
"""Trainium2 Bass kernel for nn_EquivarientScalar (segment_reduce).

Computation (reference): 2 stacked GatedEquivariant layers over N=100000
atoms (pointwise per atom), then sc = s @ out_w + out_b and a masked
segment-sum y[b] = sum_n sc[n] * batch_mask[b, n].

Strategy (2.2-2.8x faster than the fp32r baseline on HW):
  - Everything bf16: inputs, weights, intermediates. Halves HBM traffic
    (memory-target regime) at identical PE speed (1 cyc/row, same as
    fp32r at N>=256). Matmul accumulation stays fp32 in PSUM.
  - 12800 atoms/core (25 blocks of 512; 12 pairs + 1 tail block) instead
    of 13312 -- 4% less padding work everywhere.
  - Layer-2 scalar path folded on host: Wp = a2w_s0 @ a1w_s1 replaces
    the a2s matmul + bias evac; its bias lands in the layer-2 silu bias.
  - DVE q01 = sq0+sq1 runs as a stock bf16 tensor_tensor (2x mode); the
    Pool engine only does the rsqrt bit-trick seed (Pool add/mult runs at
    0.42 efficiency -- measured 1.8us per 1024-wide op -- so everything
    else sits on ACT(from PSUM: fast path, 0.46ns/elem) or DVE).
  - Norm chain per block: ACT squares c0,c1 (PSUM->SBUF bf16); DVE
    q01 add; DVE custom SQA q = c2^2 + q01 (one PSUM stream); Pool u32
    value-cast seed; DVE custom fused-Newton sqrt -> n2 bf16.
    (Pool/GPSIMD cannot touch PSUM; activation tables have no set with
    both silu and sqrt, hence the seed+Newton sqrt off ACT.)
  - Gate: one 1536-wide DVE op with a zero-stride broadcast AP over g.
  - Manual 4-stage software pipeline (front | mid1 | mid2 | tail emitted
    oldest-stage-first at offsets OFFSETS) so each engine's in-order
    queue always holds independent work from adjacent pairs; PSUM v3 tag
    rotation then references older pairs and the PE is rarely blocked.
  - Segment reduce on-chip: sc columns via h2-chunk stationary matmuls,
    y += maskT_chunk^T @ sc per 128 atoms, mask in bf16 (0/1 exact).
    Host sums the 8 per-core partial y vectors.
"""

import os
import sys

for _p in ("/opt/trn_rl_repo", "/root/.axon_site/_ro/trn_rl_repo"):
    if os.path.isdir(_p) and _p not in sys.path:
        sys.path.insert(0, _p)

os.environ.setdefault("BASS_NEVER_TRACE", "1")  # no NTFF hook in this axon build

import numpy as np

import concourse.bass as bass
import concourse.tile as tile
from concourse import bacc, mybir
from concourse import dve_ops as _dve_ops
from concourse.alu_op_type import AluOpType
from concourse.bass_utils import run_bass_kernel_spmd
from concourse.dve_ops import OPS as _DVE_OPS
from concourse.dve_ops import _CUSTOM_DVE_ROW_BASE, _SUB_OPCODE_FOR_NAME, DveOp
from concourse.dve_spec import C0 as _C0
from concourse.dve_spec import C1 as _C1
from concourse.dve_spec import Spec as _Spec
from concourse.dve_spec import Src0 as _Src0
from concourse.dve_spec import Src1 as _Src1
from concourse.dve_spec import lower as _dve_lower
from concourse.dve_spec import sq as _sq
from concourse.dve_uop import DveOpSpec as _DveOpSpec

N_CORES = 8
NA_FULL = 100000
NA_CORE = NA_FULL // N_CORES   # 12500
BLK = 512
NBLK = 25                      # 25 blocks of 512 = 12800 padded atoms/core
NA = NBLK * BLK
F = 128

F32 = mybir.dt.float32
BF16 = mybir.dt.bfloat16
U32 = mybir.dt.uint32
AF = mybir.ActivationFunctionType

W_NAMES = ["w1_0", "w2_0", "w2_1", "a1w_s0", "a1w_n0", "a1w_n1",
           "a2w_g0", "wp"]

# rsqrt magic seed (computed via u32 value-casts on Pool) + one fused
# Newton-ish stage on DVE. Constants fitted offline (baseline-validated):
# wide-range fp32 max rel err ~1e-3; q=0 -> 0 (no NaN).
MAGIC_F = 1596013007.0
SQ1_C0, SQ1_C1 = 1.6695484, 0.688087555  # n2 = (q*y0)*(C0 - C1*q*y0^2)

_last_results = None
_last_nc = None
_last_in_maps = None
ABLATE = "full"  # timing ablations: full | no_scy | no_gate | no_norm | no_mid
GATE_MODE = "bcast"  # bcast: one 1536-wide op w/ zero-stride AP; planes: 3 ops
OFFSETS = (1, 2, 3)  # software-pipeline stage offsets (mid1, mid2, tail)


def _ref_sqa(in0, in1, s0, s1, imm2):
    return (in0.astype(np.float32) * in0 + in1).astype(np.float32)


def _ref_sqrt_fin(in0, in1, s0, s1, imm2):
    qy = (in0 * in1).astype(np.float32)
    return (qy * (np.float32(s0) - np.float32(s1) * (qy * in1))).astype(np.float32)


def _register_ops():
    by_name = {op.name: op for op in _DVE_OPS}
    if "SQA_ANT" in _SUB_OPCODE_FOR_NAME and "SQRT_FIN_ANT" in _SUB_OPCODE_FOR_NAME:
        return by_name["SQA_ANT"], by_name["SQRT_FIN_ANT"]

    def make(name, body, ref):
        if name in _SUB_OPCODE_FOR_NAME:
            return by_name[name]
        op = DveOp(name, _Spec(body=body, reference=ref), subdim=False,
                   uops_sha={})
        opcode = _CUSTOM_DVE_ROW_BASE + len(_DVE_OPS)
        for ver in ("v3", "v4"):
            try:
                spec = _DveOpSpec(name=name, opcode=opcode,
                                  uops=_dve_lower(op.spec, ver=ver),
                                  rd1_en=_dve_ops.has_src1(op.spec))
                op.uops_sha[ver] = spec.sha(ver)
            except Exception:
                pass
        _SUB_OPCODE_FOR_NAME[name] = opcode
        _DVE_OPS.append(op)
        return op

    # q = c2^2 + q01        [in0 = v2_c2 (PSUM), in1 = q01 (SBUF)]
    sqa = make("SQA_ANT", _sq(_Src0) + _Src1, _ref_sqa)
    # n2 = (q*y0) * (C0 - C1 * (q*y0) * y0)   [in0 = q, in1 = y0 seed]
    _qy = _Src0 * _Src1
    fin = make("SQRT_FIN_ANT",
               _qy * (_C0 - _C1 * (_qy * _Src1)),
               _ref_sqrt_fin)
    return sqa, fin


def _build(b_fold: float, reps: int = 1, trace_sim: bool = False):
    # reps > 1 repeats the whole computation inside one NEFF (timing only --
    # y then accumulates reps copies; used to subtract host/tunnel overhead).
    OP_SQA, OP_FIN = _register_ops()
    nc = bacc.Bacc("TRN2", target_bir_lowering=False)

    # packed input, partition-major: row f = [s 512 | v 3*512 | mask 4*128]
    xp = nc.dram_tensor("xpack", (F, NBLK, 5 * BLK), BF16, kind="ExternalInput")
    wd = {n: nc.dram_tensor(n, (F, F), BF16, kind="ExternalInput") for n in W_NAMES}
    # (F, 2): col 0 = a2w_s1 @ out_w, col 1 = zeros
    wf = nc.dram_tensor("w_fold", (F, 2), BF16, kind="ExternalInput")
    a1b0 = nc.dram_tensor("a1b0", (F, 1), F32, kind="ExternalInput")
    b2p = nc.dram_tensor("b2p", (F, 1), F32, kind="ExternalInput")
    a2bg = nc.dram_tensor("a2bg", (F, 1), F32, kind="ExternalInput")
    y = nc.dram_tensor("y", (F, 1), F32, kind="ExternalOutput")

    # 12 pairs + tail block
    groups = [[2 * i, 2 * i + 1] for i in range(12)] + [[24]]

    with tile.TileContext(nc, trace_sim=trace_sim) as tc:
        # PSUM budget (8 banks): v3 tag 3 banks x2 bufs + "a" tag 1 bank x2.
        with tc.tile_pool(name="wpool", bufs=1) as wp, \
             tc.tile_pool(name="io", bufs=6) as io, \
             tc.tile_pool(name="work", bufs=2) as wk, \
             tc.tile_pool(name="workx", bufs=5) as wkx, \
             tc.tile_pool(name="psv", bufs=2, space="PSUM") as psv, \
             tc.tile_pool(name="psa", bufs=2, space="PSUM") as psa:

            wt = {}
            for n in W_NAMES:
                wt[n] = wp.tile([F, F], BF16, name=n, tag=n)
                nc.sync.dma_start(out=wt[n], in_=wd[n][:, :])
            wft = wp.tile([F, 2], BF16, tag="wf")
            nc.sync.dma_start(out=wft, in_=wf[:, :])
            bt = {}
            for n, d in [("a1b0", a1b0), ("b2p", b2p), ("a2bg", a2bg)]:
                bt[n] = wp.tile([F, 1], F32, name=n, tag=n)
                nc.sync.dma_start(out=bt[n], in_=d[:, :])

            y_sb = wp.tile([F, 1], F32, tag="y_sb")
            nc.vector.memset(y_sb, 0.0)

            def norm_front(xts_or_vouts, nh, tag, from_xt):
                """v-matmuls + squares for nh blocks; returns (v3s, q, n2P).
                Emits: per h: 1 matmul (N=1536) + 1 ACT square (c01) +
                1 DVE SQA; per group: 1 Pool add, 1 Pool seed, 1 DVE fin."""
                w = wt["w2_0"] if tag == "l1" else wt["w2_1"]
                sqP = (None if ABLATE in ("no_norm", "pe_only")
                       else wk.tile([F, nh, 2, BLK], BF16, tag=f"sqP_{tag}"))
                v3s = []
                for h in range(nh):
                    v3 = psv.tile([F, 3, BLK], F32, tag="v3")
                    for c in range(3):
                        if from_xt:
                            rhs = xts_or_vouts[:, h,
                                               (1 + c) * BLK:(2 + c) * BLK]
                        else:
                            rhs = xts_or_vouts[h][:, c, :]
                        nc.tensor.matmul(v3[:, c, :], w, rhs)
                    if ABLATE not in ("no_norm", "pe_only"):
                        nc.scalar.activation(out=sqP[:, h, :, :],
                                             in_=v3[:, 0:2, :], func=AF.Square)
                    v3s.append(v3)
                if ABLATE in ("no_norm", "pe_only"):
                    n2P = wkx.tile([F, nh, BLK], BF16, tag=f"n2P_{tag}")
                    nc.vector.memset(n2P, 0.5)
                    return v3s, n2P
                q01P = wk.tile([F, nh, BLK], BF16, tag=f"q01P_{tag}")
                nc.vector.tensor_tensor(out=q01P, in0=sqP[:, :, 0, :],
                                        in1=sqP[:, :, 1, :], op=AluOpType.add)
                q = wk.tile([F, nh, BLK], F32, tag=f"q_{tag}")
                for h in range(nh):
                    nc.vector._custom_dve(OP_SQA, out=q[:, h, :],
                                          in0=v3s[h][:, 2, :],
                                          in1=q01P[:, h, :])
                sd = wk.tile([F, nh, BLK], F32, tag=f"sd_{tag}")
                nc.gpsimd.tensor_scalar(out=sd.bitcast(U32),
                                        in0=q.bitcast(U32),
                                        scalar1=-0.5, scalar2=MAGIC_F,
                                        op0=AluOpType.mult, op1=AluOpType.add)
                n2P = wkx.tile([F, nh, BLK], BF16, tag=f"n2P_{tag}")
                nc.vector._custom_dve(OP_FIN, out=n2P, in0=q, in1=sd,
                                      s0=SQ1_C0, s1=SQ1_C1)
                return v3s, n2P

            # Manual 3-stage software pipeline: iteration i emits
            # front(i) | mid(i-1) | tail(i-2), so each engine always has
            # independent work from adjacent pairs in its in-order queue.
            ctxs = {}

            def st_front(it):
                hs = groups[it % len(groups)]
                nh = len(hs)
                b0 = hs[0]
                xt = io.tile([F, nh, 5 * BLK], BF16, tag="xt")
                with tc.high_priority(offset=110):
                    nc.sync.dma_start(out=xt, in_=xp[:, b0:b0 + nh, :])
                _, n2P1 = norm_front(xt, nh, "l1", True)
                ctxs[it] = {"nh": nh, "xt": xt, "n2P1": n2P1}

            def st_mid(it):
                cx = ctxs[it]
                nh, xt, n2P1 = cx["nh"], cx["xt"], cx["n2P1"]
                if ABLATE == "no_mid":
                    h1P = wkx.tile([F, nh, BLK], BF16, tag="h1P")
                    nc.vector.memset(h1P, 0.5)
                    cx["h1P"], cx["vouts"] = h1P, None
                    return
                h1P = wkx.tile([F, nh, BLK], BF16, tag="h1P")
                gP = wk.tile([F, nh, BLK], BF16, tag="gP")
                vouts = []
                for h in range(nh):
                    a1 = psa.tile([F, BLK], F32, tag="a")
                    nc.tensor.matmul(a1, wt["a1w_s0"], xt[:, h, 0:BLK],
                                     start=True, stop=False)
                    nc.tensor.matmul(a1, wt["a1w_n0"], n2P1[:, h, :],
                                     start=False, stop=True)
                    nc.scalar.activation(out=h1P[:, h, :], in_=a1,
                                         func=AF.Silu, bias=bt["a1b0"])

                    a2g = psa.tile([F, BLK], F32, tag="a")
                    nc.tensor.matmul(a2g, wt["a2w_g0"], h1P[:, h, :])
                    nc.scalar.activation(out=gP[:, h, :], in_=a2g,
                                         func=AF.Identity, bias=bt["a2bg"])

                    # v1 matmuls + one 1536-wide gate via broadcast AP
                    v13 = psv.tile([F, 3, BLK], F32, tag="v3")
                    for c in range(3):
                        nc.tensor.matmul(v13[:, c, :], wt["w1_0"],
                                         xt[:, h, (1 + c) * BLK:(2 + c) * BLK])
                    g = gP[:, h, :]
                    g_bc3 = bass.AP(tensor=g.tensor, offset=g.offset,
                                    ap=[g.ap[0], [0, 3], g.ap[1]])
                    vout = wk.tile([F, 3, BLK], BF16, tag="vout")
                    if ABLATE in ("no_gate", "pe_only"):
                        nc.vector.memset(vout, 0.5)
                    elif GATE_MODE == "bcast":
                        nc.vector.tensor_tensor(out=vout, in0=v13, in1=g_bc3,
                                                op=AluOpType.mult)
                    else:
                        for c in range(3):
                            nc.vector.tensor_tensor(out=vout[:, c, :],
                                                    in0=v13[:, c, :], in1=g,
                                                    op=AluOpType.mult)
                    vouts.append(vout)

                cx["h1P"], cx["vouts"] = h1P, vouts

            def st_mid2(it):
                cx = ctxs[it]
                nh = cx["nh"]
                if ABLATE == "no_mid":
                    n2P2 = wkx.tile([F, nh, BLK], BF16, tag="n2P_l2")
                    nc.vector.memset(n2P2, 0.5)
                    cx["n2P2"] = n2P2
                    return
                _, n2P2 = norm_front(cx.pop("vouts"), nh, "l2", False)
                cx["n2P2"] = n2P2

            def st_tail(it):
                cx = ctxs.pop(it)
                nh, xt, h1P, n2P2 = cx["nh"], cx["xt"], cx["h1P"], cx["n2P2"]
                h2P = wk.tile([F, nh, BLK], BF16, tag="h2P")
                for h in range(nh):
                    a1b_ = psa.tile([F, BLK], F32, tag="a")
                    nc.tensor.matmul(a1b_, wt["wp"], h1P[:, h, :],
                                     start=True, stop=False)
                    nc.tensor.matmul(a1b_, wt["a1w_n1"], n2P2[:, h, :],
                                     start=False, stop=True)
                    nc.scalar.activation(out=h2P[:, h, :], in_=a1b_,
                                         func=AF.Silu, bias=bt["b2p"])

                if ABLATE in ("no_scy",):
                    return
                sc_ps = psa.tile([F, nh, BLK // F, 2], F32, tag="a")
                for h in range(nh):
                    for k in range(BLK // F):
                        nc.tensor.matmul(sc_ps[:, h, k, :],
                                         h2P[:, h, k * F:(k + 1) * F], wft)
                sc_sb = wk.tile([F, nh, BLK // F, 2], BF16, tag="sc_sb")
                nc.vector.tensor_scalar(out=sc_sb, in0=sc_ps,
                                        scalar1=float(b_fold),
                                        scalar2=None, op0=AluOpType.add)
                y_ps = psa.tile([F, 2], F32, tag="a")
                nmm = 0
                nmm_tot = nh * (BLK // F)
                for h in range(nh):
                    mT_t = xt[:, h, 4 * BLK:].rearrange("p (k g) -> p k g",
                                                        k=BLK // F)
                    for k in range(BLK // F):
                        nc.tensor.matmul(y_ps, mT_t[:, k, :],
                                         sc_sb[:, h, k, :],
                                         start=(nmm == 0),
                                         stop=(nmm == nmm_tot - 1),
                                         skip_group_check=True)
                        nmm += 1
                nc.vector.tensor_tensor(out=y_sb, in0=y_sb, in1=y_ps[:, 0:1],
                                        op=AluOpType.add)

            o1, o2, o3 = OFFSETS
            total = len(groups) * reps
            for it in range(total + o3):
                if it < total:
                    st_front(it)
                if it >= o3:
                    st_tail(it - o3)
                if o2 <= it < total + o2:
                    st_mid2(it - o2)
                if o1 <= it < total + o1:
                    st_mid(it - o1)

            nc.sync.dma_start(out=y[:, :], in_=y_sb)

    nc.finalize()
    return nc


def kernel(s, v, r, batch_mask, w1, w2, a1w, a1b, a2w, a2b, out_w, out_b):
    global _last_results
    del r  # unused by the reference computation

    s = np.ascontiguousarray(np.asarray(s, dtype=np.float32)).reshape(NA_FULL, F)
    v = np.ascontiguousarray(np.asarray(v, dtype=np.float32)).reshape(NA_FULL, 3, F)
    batch_mask = np.ascontiguousarray(
        np.asarray(batch_mask, dtype=np.float32)).reshape(F, NA_FULL)
    w1 = np.asarray(w1, dtype=np.float64)
    w2 = np.asarray(w2, dtype=np.float64)
    a1w = np.asarray(a1w, dtype=np.float64)
    a1b = np.asarray(a1b, dtype=np.float64)
    a2w = np.asarray(a2w, dtype=np.float64)
    a2b = np.asarray(a2b, dtype=np.float64)
    out_w = np.asarray(out_w, dtype=np.float64)
    out_b = np.asarray(out_b, dtype=np.float64)
    assert w1.shape == (2, F, F), "kernel is specialized to L=2"

    bf16 = mybir.dt.np(BF16)

    # folded final projection: sc = h2 @ (a2w_s1 @ out_w) + b_fold
    w_fold = np.zeros((F, 2), dtype=np.float64)
    w_fold[:, 0:1] = a2w[1][:, :F] @ out_w
    b_fold = float(a2b[1][:F] @ out_w[:, 0] + out_b[0])

    # folded layer-2 scalar path: a1w_s1^T s1 = (a2w_s0 @ a1w_s1)^T h1 + const
    wp = a2w[0][:, :F] @ a1w[1][:F, :]
    b2p = a1b[1] + a1w[1][:F, :].T @ a2b[0][:F]

    weights = {
        "w1_0": w1[0], "w2_0": w2[0], "w2_1": w2[1],
        "a1w_s0": a1w[0][:F, :], "a1w_n0": a1w[0][F:, :],
        "a1w_n1": a1w[1][F:, :], "a2w_g0": a2w[0][:, F:], "wp": wp,
    }
    weights = {k: np.ascontiguousarray(a, dtype=bf16)
               for k, a in weights.items()}
    w_fold16 = np.ascontiguousarray(w_fold, dtype=bf16)
    bias_cols = {
        "a1b0": np.ascontiguousarray(a1b[0].reshape(F, 1), dtype=np.float32),
        "b2p": np.ascontiguousarray(b2p.reshape(F, 1), dtype=np.float32),
        "a2bg": np.ascontiguousarray(a2b[0][F:].reshape(F, 1),
                                     dtype=np.float32),
    }

    in_maps = []
    for c in range(N_CORES):
        sl = slice(c * NA_CORE, (c + 1) * NA_CORE)
        sT = np.zeros((F, NA), dtype=np.float32)
        sT[:, :NA_CORE] = s[sl].T
        vT = np.zeros((F, 3, NA), dtype=np.float32)
        vT[:, :, :NA_CORE] = v[sl].transpose(2, 1, 0)
        mT = np.zeros((NA, F), dtype=np.float32)
        mT[:NA_CORE] = batch_mask[:, sl].T
        xp = np.empty((F, NBLK, 5 * BLK), dtype=np.float32)
        xp[:, :, 0:BLK] = sT.reshape(F, NBLK, BLK)
        xp[:, :, BLK:4 * BLK] = (
            vT.reshape(F, 3, NBLK, BLK).transpose(0, 2, 1, 3)
            .reshape(F, NBLK, 3 * BLK))
        xp[:, :, 4 * BLK:] = (
            mT.reshape(NBLK, BLK // F, F, F).transpose(2, 0, 1, 3)
            .reshape(F, NBLK, BLK))
        m = {"xpack": np.ascontiguousarray(xp, dtype=bf16),
             "w_fold": w_fold16}
        m.update(weights)
        m.update(bias_cols)
        in_maps.append(m)

    nc = _build(b_fold)
    res = run_bass_kernel_spmd(nc, in_maps, core_ids=list(range(N_CORES)))
    global _last_nc, _last_in_maps
    _last_results, _last_nc, _last_in_maps = res, nc, in_maps

    yv = np.zeros((F, 1), dtype=np.float64)
    for c in range(N_CORES):
        yv += res.results[c]["y"].astype(np.float64)
    return yv.astype(np.float32)
